# revision 1
# baseline (speedup 1.0000x reference)
"""Trainium2 Bass kernel for nn_AttentionOnDetail (sparse patch attention).

Data-parallel over batch B=8 across 8 NeuronCores; one batch per core.
Per core:
  phase 1: stream x[b] (4MB) in patch-major tiles [128 patches, 2048];
           per-patch sum-of-squares via ScalarE activation(Square,
           accum_out) and dot(patch, patch_w) via VectorE
           tensor_tensor_reduce -> 512 logits.
  top-4:   top-8 values -> 4th value threshold -> mask * (512-i) ->
           max_index returns the 4 selected patch ids ascending;
           expand to 64 token ids; indirect DMA gathers x_sel.
  phase 2: qkvg projection of only the 64 selected tokens (the
           reference computes all 8192), DRAM-bounce rearrange into
           q/k/v/g, RoPE + rmsnorm + tao, causal attention over
           65 rows (sink + 64), sigmoid gating, output projection.
"""

import sys
import numpy as np

for _p in ("/opt/trn_rl_repo",):
    if _p not in sys.path:
        sys.path.insert(0, _p)

import concourse.bass as bass
import concourse.bacc as bacc
import concourse.tile as tile
from concourse import mybir
from concourse.bass_utils import run_bass_kernel_spmd

F32 = mybir.dt.float32
I32 = mybir.dt.int32
U32 = mybir.dt.uint32
AF = mybir.ActivationFunctionType
ALU = mybir.AluOpType
AX = mybir.AxisListType

B, T, C, H, T0 = 8, 8192, 128, 8, 16
NP = T // T0          # 512 patches
PATCH = T0 * C        # 2048 elements per patch
S = 65                # sink + 64 selected tokens
NSEL = 64
FQ = 4 * C * H        # 4096
EPS = 1.1920929e-07
SCALE = 1.0 / float(np.sqrt(np.float32(C)))
NEG_BIG = -1.0e30


def rap(t, apl, offset=0):
    """Raw AP over a tile/AP's storage, flat element strides.

    For SBUF tensors the partition step of dim0 equals the tensor's
    free size per partition.
    """
    base = t if isinstance(t, bass.AP) else t[:]
    return bass.AP(tensor=base.tensor, offset=base.offset + offset,
                   ap=[list(x) for x in apl])


def build_kernel(nc):
    xb = nc.dram_tensor("xb", [T, C], F32, kind="ExternalInput")
    pw = nc.dram_tensor("pw", [PATCH], F32, kind="ExternalInput")
    wqkvg = nc.dram_tensor("wqkvg", [FQ, C], F32, kind="ExternalInput")
    wout = nc.dram_tensor("wout", [C, H * C], F32, kind="ExternalInput")
    sink = nc.dram_tensor("sink", [H, C], F32, kind="ExternalInput")
    cosd = nc.dram_tensor("cosd", [S, 64], F32, kind="ExternalInput")
    sind = nc.dram_tensor("sind", [S, 64], F32, kind="ExternalInput")
    tao = nc.dram_tensor("tao", [S, 2], F32, kind="ExternalInput")
    ident = nc.dram_tensor("ident", [128, 128], F32, kind="ExternalInput")
    off16 = nc.dram_tensor("off16", [NSEL, 1], F32, kind="ExternalInput")
    negio = nc.dram_tensor("negio", [1, NP], F32, kind="ExternalInput")
    cmask = nc.dram_tensor("cmask", [S, S], F32, kind="ExternalInput")
    repmat = nc.dram_tensor("repmat", [4, NSEL], F32, kind="ExternalInput")
    out = nc.dram_tensor("out", [NSEL, C], F32, kind="ExternalOutput")

    with tile.TileContext(nc) as tc:
        _emit(tc, nc, xb, pw, wqkvg, wout, sink, cosd, sind, tao, ident,
              off16, negio, cmask, repmat, out)
    return nc


def _emit(tc, nc, xb, pw, wqkvg, wout, sink, cosd, sind, tao, ident,
          off16, negio, cmask, repmat, out):
    import os
    LEVEL = int(os.environ.get("KLEVEL", "9"))
    from contextlib import ExitStack
    ctx = ExitStack()
    with ctx:
        const1 = ctx.enter_context(tc.tile_pool(name="const1", bufs=1))
        xpool = ctx.enter_context(tc.tile_pool(name="xpool", bufs=3))
        junkp = ctx.enter_context(tc.tile_pool(name="junkp", bufs=1))
        stat = ctx.enter_context(tc.tile_pool(name="stat", bufs=4))
        sb = ctx.enter_context(tc.tile_pool(name="sb", bufs=1))
        psall = ctx.enter_context(tc.tile_pool(name="psall", bufs=1,
                                                space="PSUM"))
        # one tile owning all 8 PSUM banks; regions are choreographed
        # manually (same-tile writes avoid slot-realloc wait explosions)
        PS = psall.tile([128, 4096], F32)
        # bank layout (f32 offsets):
        #   0:512     transpose slot A
        #   512:1024  transpose slot B
        #   1024:2048 qkvg matmul groups
        #   2048:2560 logits-T / repmat / x_selT / out
        #   2560:3584 att / y
        #   3584:4096 junk absorber columns
        dramp = ctx.enter_context(tc.tile_pool(name="dramp", bufs=1,
                                               space="DRAM"))
        # DRAM scratch: natural qkvg copy [64 tokens, 4096], then
        # per-tensor row-permuted copy [4, 64, 1024]
        qperm = dramp.tile([NSEL, FQ], F32)
        qperm2 = dramp.tile([4, S, H * C], F32)

        # ---------------- constants / weight prep ----------------
        ident_t = const1.tile([128, 128], F32)
        nc.sync.dma_start(out=ident_t[:, :], in_=ident[:, :])

        # pw broadcast to 128 partitions via K=1 matmul (SWDGE step-0
        # partition-broadcast DMA crashes the exec unit)
        pw_sb = const1.tile([1, PATCH], F32)
        nc.sync.dma_start(out=pw_sb[:, :], in_=rap(pw[:], [[1, 1], [1, PATCH]]))
        ones_t = const1.tile([1, 128], F32)
        nc.vector.memset(ones_t[:, :], 1.0)
        pwB = const1.tile([128, PATCH], F32)

        # absorb the ident_t DMA wait (every PE matmul may carry at most
        # ONE sync wait -- walrus funnels waits into the LDW struct)
        nc.tensor.matmul(out=PS[:, 3584:3585], lhsT=ident_t[:, :],
                         rhs=ident_t[:, 0:1], start=True, stop=True)

        # absorb pw's DMA lane, then broadcast pw into 128 partitions
        nc.tensor.matmul(out=PS[0:128, 3585:3586], lhsT=pw_sb[0:1, 0:128],
                         rhs=pw_sb[0:1, 0:1], start=True, stop=True)
        for q4 in range(4):
            pbase = 1024 + 512 * (q4 % 2) if q4 < 2 else 2560 + 512 * (q4 % 2)
            nc.tensor.matmul(out=PS[:, pbase:pbase + 512],
                             lhsT=ones_t[:, :],
                             rhs=pw_sb[:, 512 * q4:512 * (q4 + 1)],
                             start=True, stop=True)
            nc.scalar.copy(out=pwB[:, 512 * q4:512 * (q4 + 1)],
                           in_=PS[:, pbase:pbase + 512])

        # W_qkvg natural: w_nat[p, t, c] = W[t*128+p, c]
        w_nat = const1.tile([128, 32, C], F32)
        nc.sync.dma_start(
            out=w_nat[:, :, :],
            in_=rap(wqkvg[:, :], [[C, 128], [128 * C, 32], [1, C]]))
        # wqT[:, t, :] = W[t*128:(t+1)*128, :].T   (c-major)
        wqT = const1.tile([128, 32, C], F32)
        for g in range(8):
            base = 512 * (g % 2)
            for j in range(4):
                nc.tensor.matmul(
                    out=PS[:, base + j * 128:base + (j + 1) * 128],
                    lhsT=w_nat[:, 4 * g + j, :], rhs=ident_t[:, :],
                    start=True, stop=True)
            nc.vector.tensor_copy(
                out=wqT[:, 4 * g:4 * g + 4, :],
                in_=PS[:, base:base + 512].rearrange(
                    "p (a b) -> p a b", a=4))

        wo_nat = const1.tile([128, H, C], F32)
        nc.sync.dma_start(
            out=wo_nat[:, :, :],
            in_=rap(wout[:, :], [[H * C, 128], [128, H], [1, C]]))
        # absorb wo_nat's DMA wait on PE
        nc.tensor.matmul(out=PS[:, 3586:3587], lhsT=wo_nat[:, 0, :],
                         rhs=ident_t[:, 0:1], start=True, stop=True)
        woT = const1.tile([128, H, C], F32)
        for g in range(2):
            base = 512 * (g % 2)
            for j in range(4):
                nc.tensor.matmul(
                    out=PS[:, base + j * 128:base + (j + 1) * 128],
                    lhsT=wo_nat[:, 4 * g + j, :], rhs=ident_t[:, :],
                    start=True, stop=True)
            nc.vector.tensor_copy(
                out=woT[:, 4 * g:4 * g + 4, :],
                in_=PS[:, base:base + 512].rearrange(
                    "p (a b) -> p a b", a=4))

        eps_t = const1.tile([128, 1], F32)
        nc.vector.memset(eps_t[:, :], EPS)

        # seq-position permutation: partition p<64 = position p+1,
        # partition 64 = position 0 (sink)
        cos_t = const1.tile([S, 64], F32)
        nc.sync.dma_start(out=cos_t[0:NSEL, :], in_=cosd[1:S, :])
        nc.sync.dma_start(out=cos_t[NSEL:S, :], in_=cosd[0:1, :])
        sinD = const1.tile([S, 2, 64], F32)   # [:,0,:]=+sin  [:,1,:]=-sin
        nc.sync.dma_start(out=sinD[0:NSEL, 0, :], in_=sind[1:S, :])
        nc.sync.dma_start(out=sinD[NSEL:S, 0, :], in_=sind[0:1, :])
        nc.scalar.mul(out=sinD[0:NSEL, 1, :], in_=sinD[0:NSEL, 0, :],
                      mul=-1.0)
        nc.scalar.mul(out=sinD[NSEL:S, 1, :], in_=sinD[NSEL:S, 0, :],
                      mul=-1.0)

        taob = const1.tile([S, 2], F32)
        nc.sync.dma_start(out=taob[:, :], in_=tao[:, :])

        off16_t = const1.tile([NSEL, 1], F32)
        nc.sync.dma_start(out=off16_t[:, :], in_=off16[:, :])
        repmat_t = const1.tile([4, NSEL], F32)
        nc.sync.dma_start(out=repmat_t[:, :], in_=repmat[:, :])
        # absorb repmat_t's DMA wait on PE early
        nc.tensor.matmul(out=PS[0:NSEL, 3585:3586], lhsT=repmat_t[:, :],
                         rhs=repmat_t[:, 0:1], start=True, stop=True)
        negio_t = const1.tile([1, NP], F32)
        nc.sync.dma_start(out=negio_t[:, :], in_=negio[:, :])
        cmask_t = const1.tile([S, S], F32)
        nc.sync.dma_start(out=cmask_t[:, :], in_=cmask[:, :])

        # ---------------- phase 1: per-patch stats ----------------
        touch = const1.tile([128, 24], F32)
        nc.vector.tensor_copy(out=touch[:, 1:2], in_=pwB[:, 0:1])
        # absorb every constant table's DMA lane on DVE early (cheap,
        # off the critical path)
        nc.vector.tensor_copy(out=touch[0:NSEL, 2:3], in_=cos_t[0:NSEL, 0:1])
        nc.vector.tensor_copy(out=touch[NSEL:S, 3:4], in_=cos_t[NSEL:S, 0:1])
        nc.vector.tensor_copy(out=touch[0:NSEL, 4:5],
                              in_=sinD[0:NSEL, 0, 0:1])
        nc.vector.tensor_copy(out=touch[NSEL:S, 5:6],
                              in_=sinD[NSEL:S, 0, 0:1])
        nc.vector.tensor_copy(out=touch[0:S, 6:7], in_=cmask_t[:, 0:1])
        nc.vector.tensor_copy(out=touch[0:S, 7:8], in_=taob[:, 0:1])
        nc.vector.tensor_copy(out=touch[0:1, 8:9], in_=negio_t[:, 0:1])
        nc.vector.tensor_copy(out=touch[0:NSEL, 9:10], in_=off16_t[:, :])
        logits_col = stat.tile([128, 4], F32)
        for i in range(4):
            xp = xpool.tile([128, PATCH], F32, tag="xp")
            nc.sync.dma_start(
                out=xp[:, :],
                in_=rap(xb[:, :], [[PATCH, 128], [1, PATCH]],
                        offset=i * 128 * PATCH))
            junk = junkp.tile([128, PATCH], F32, tag="junk")
            ss = stat.tile([128, 1], F32, tag="ss")
            nc.scalar.activation(out=junk[:, :], in_=xp[:, :], func=AF.Square,
                                 accum_out=ss[:, :])
            junk2 = junkp.tile([128, PATCH], F32, tag="junk2")
            dotv = stat.tile([128, 1], F32, tag="dotv")
            nc.vector.scalar_tensor_tensor(
                out=junk2[:, :], in0=xp[:, :], scalar=1.0, in1=pwB[:, :],
                op0=ALU.mult, op1=ALU.mult, accum_out=dotv[:, :])
            sq = stat.tile([128, 1], F32, tag="sq")
            nc.scalar.activation(out=sq[:, :], in_=ss[:, :], func=AF.Sqrt,
                                 bias=eps_t[0:128, 0:1], scale=1.0 / PATCH)
            rs = stat.tile([128, 1], F32, tag="rs")
            nc.vector.reciprocal(out=rs[:, :], in_=sq[:, :])
            nc.vector.tensor_mul(logits_col[:, i:i + 1], dotv[:, :], rs[:, :])

        # one row [1, 512]: PE-transpose [128, 4] -> [4, 128], then a
        # contiguous SBUF->SBUF DMA into [1, 512]
        nc.tensor.matmul(out=PS[0:4, 2048:2176], lhsT=logits_col[:, :],
                         rhs=ident_t[:, :], start=True, stop=True)
        lrow4 = stat.tile([4, 128], F32)
        nc.scalar.copy(out=lrow4[:, :], in_=PS[0:4, 2048:2176])
        logits_row = stat.tile([1, NP], F32)
        nc.sync.dma_start(out=logits_row[:, :], in_=lrow4[:, :])

        if LEVEL == 1:
            nc.sync.dma_start(out=out[:, 0:4], in_=logits_col[0:64, :])
            return
        # ---------------- top-4 selection ----------------
        max8 = stat.tile([1, 8], F32)
        nc.vector.max(out=max8[:, :], in_=logits_row[:, :])
        mask = stat.tile([1, NP], F32)
        nc.vector.tensor_scalar(out=mask[:, :], in0=logits_row[:, :],
                                scalar1=max8[:, 3:4], scalar2=None,
                                op0=ALU.is_ge)
        masked = stat.tile([1, NP], F32)
        nc.vector.tensor_mul(masked[:, :], mask[:, :], negio_t[:, :])
        mm8 = stat.tile([1, 8], F32)
        nc.vector.max(out=mm8[:, :], in_=masked[:, :])
        idx8 = stat.tile([1, 8], U32)
        nc.vector.max_index(out=idx8[:, :], in_max=mm8[:, :],
                            in_values=masked[:, :])
        idxf = stat.tile([1, 8], F32)
        nc.vector.tensor_copy(out=idxf[:, :], in_=idx8[:, :])

        # token ids: move ids to a column via tiny DMA, then matmul with
        # the replication matrix repmat[k, m] = 16*(m//16 == k)
        idxc = stat.tile([4, 1], F32)
        nc.sync.dma_start(out=idxc[:, :], in_=idxf[0:1, 0:4])
        # absorb idxc's DMA wait
        nc.tensor.matmul(out=PS[0:1, 3587:3588], lhsT=idxc[:, :],
                         rhs=idxc[:, :], start=True, stop=True)
        nc.tensor.matmul(out=PS[0:NSEL, 2048:2049], lhsT=repmat_t[:, :],
                         rhs=idxc[:, :], start=True, stop=True)
        nc.vector.tensor_copy(out=touch[0:4, 10:11], in_=lrow4[:, 0:1])
        tok_f = stat.tile([NSEL, 1], F32)
        nc.vector.tensor_add(out=tok_f[:, :], in0=PS[0:NSEL, 2048:2049],
                             in1=off16_t[:, :])
        tok_i = stat.tile([NSEL, 1], I32)
        nc.vector.tensor_copy(out=tok_i[:, :], in_=tok_f[:, :])

        x_sel = sb.tile([NSEL, C], F32, tag="x_sel")
        nc.gpsimd.indirect_dma_start(
            out=x_sel[:, :], out_offset=None, in_=xb[:, :],
            in_offset=bass.IndirectOffsetOnAxis(ap=tok_i[:, 0:1], axis=0))

        if LEVEL == 2:
            nc.sync.dma_start(out=out[:, :], in_=x_sel[:, :])
            return
        # ---------------- qkvg projection (64 tokens) ----------------
        # absorb x_sel's (indirect) DMA wait
        nc.tensor.matmul(out=PS[:, 3588:3589], lhsT=x_sel[:, :],
                         rhs=ident_t[0:NSEL, 0:1], start=True, stop=True)
        nc.tensor.matmul(out=PS[:, 2048:2048 + NSEL], lhsT=x_sel[:, :],
                         rhs=ident_t[0:NSEL, 0:NSEL], start=True, stop=True)
        nc.scalar.copy(out=touch[0:NSEL, 11:12], in_=tok_f[:, :])
        x_selT = sb.tile([128, NSEL], F32, tag="x_selT")
        nc.scalar.copy(out=x_selT[:, :], in_=PS[:, 2048:2048 + NSEL])

        qkvg_sb = sb.tile([NSEL, FQ], F32, tag="qkvg")
        for grp in range(4):
            for j in range(2):
                k = grp * 2 + j
                nc.tensor.matmul(
                    out=PS[0:NSEL, 1024 + j * 512:1024 + (j + 1) * 512],
                    lhsT=x_selT[:, :],
                    rhs=wqT[:, 4 * k:4 * k + 4, :], start=True, stop=True)
            nc.scalar.copy(out=qkvg_sb[:, 1024 * grp:1024 * (grp + 1)],
                           in_=PS[0:NSEL, 1024:2048])

        # plain copy to DRAM; the q/k/v/g layout is an identity map in
        # flat bytes (token row 4096 = 4 dst rows of 1024)
        nc.sync.dma_start(out=qperm[:, :], in_=qkvg_sb[:, :])

        # q/k/v/g token-major [*, 8, 128] (contiguous reads)
        q_all = sb.tile([S, H, C], F32, tag="q_all")
        k_all = sb.tile([S, H, C], F32, tag="k_all")
        v_all = sb.tile([S, H, C], F32, tag="v_all")
        g_all = sb.tile([NSEL, H, C], F32, tag="g_all")
        tc.strict_bb_all_engine_barrier()
        # DRAM->DRAM row permutation into final order, sink appended
        qperm_v = qperm[:, :].rearrange("(a b) f -> a b f", b=16)
        for tens in range(4):
            joff = 4 * tens
            nc.sync.dma_start(
                out=qperm2[tens, 0:NSEL, :],
                in_=qperm_v[:, joff:joff + 4, :])
            if tens < 3:
                nc.sync.dma_start(
                    out=qperm2[tens, NSEL:S, :],
                    in_=rap(sink[:, :], [[0, 1], [1, H * C]]))
        tc.strict_bb_all_engine_barrier()
        # absorb the barrier semaphore on PE, DVE and ACT
        nc.tensor.matmul(out=PS[:, 3589:3590], lhsT=ident_t[:, :],
                         rhs=ident_t[:, 0:1], start=True, stop=True)
        nc.vector.tensor_copy(out=touch[:, 12:13], in_=eps_t[:, :])
        nc.scalar.copy(out=touch[0:1, 13:14], in_=eps_t[0:1, 0:1])
        # one contiguous readback per tensor (single DMA lane each)
        for tens, dst in enumerate((q_all, k_all, v_all, g_all)):
            ns = S if tens < 3 else NSEL
            nc.sync.dma_start(
                out=dst[0:ns, :, :],
                in_=qperm2[tens, 0:ns, :].rearrange("s (h c) -> s h c", h=H))

        if LEVEL == 3:
            nc.sync.dma_start(out=out[:, :], in_=q_all[0:NSEL, 0, :])
            return
        # ---------------- RoPE + rmsnorm + tao ----------------
        def rope_norm(src, dst, tao_col):
            r = sb.tile([S, H, C], F32, tag="rope_r")
            cos_b = cos_t[:, :].rearrange(
                "s (a b c2) -> s a b c2", a=1, b=1).to_broadcast([S, H, 2, 64])
            nc.vector.tensor_tensor(
                out=r[:, :, :].rearrange("s h (k c) -> s h k c", k=2),
                in0=src[:, :, :].rearrange("s h (k c) -> s h k c", k=2),
                in1=cos_b, op=ALU.mult)
            tmp = sb.tile([S, H, C], F32, tag="rope_t")
            # tmp_lo = q_hi * sin ; tmp_hi = q_lo * (-sin)
            nc.vector.tensor_tensor(
                out=tmp[:, :, 0:64], in0=src[:, :, 64:128],
                in1=sinD[:, 0:1, :].to_broadcast([S, H, 64]), op=ALU.mult)
            nc.vector.tensor_tensor(
                out=tmp[:, :, 64:128], in0=src[:, :, 0:64],
                in1=sinD[:, 1:2, :].to_broadcast([S, H, 64]), op=ALU.mult)
            nc.vector.tensor_add(out=r[:, :, :], in0=r[:, :, :],
                                 in1=tmp[:, :, :])
            sqq = sb.tile([S, H, C], F32, tag="rope_sq")
            nc.scalar.activation(out=sqq[:, :, :], in_=r[:, :, :],
                                 func=AF.Square)
            ssq = sb.tile([S, H], F32, tag="rope_ss")
            nc.vector.tensor_reduce(out=ssq[:, :], in_=sqq[:, :, :],
                                    axis=AX.X, op=ALU.add)
            sf = sb.tile([S, H], F32, tag="rope_sf")
            nc.scalar.activation(out=sf[:, :], in_=ssq[:, :], func=AF.Sqrt,
                                 bias=eps_t[0:S, 0:1], scale=1.0 / C)
            rf = sb.tile([S, H], F32, tag="rope_rf")
            nc.vector.reciprocal(out=rf[:, :], in_=sf[:, :])
            nc.vector.tensor_scalar_mul(rf[:, :], rf[:, :], tao_col)
            nc.vector.tensor_tensor(
                out=dst[:, :, :], in0=r[:, :, :],
                in1=rf[:, :].rearrange("s (h a) -> s h a", a=1)
                    .to_broadcast([S, H, C]), op=ALU.mult)

        qn = sb.tile([S, H, C], F32, tag="qn")
        kn = sb.tile([S, H, C], F32, tag="kn")
        rope_norm(q_all, qn, taob[:, 0:1])
        rope_norm(k_all, kn, taob[:, 1:2])

        if LEVEL == 4:
            nc.sync.dma_start(out=out[:, :], in_=qn[0:NSEL, 0, :])
            return
        # ---------------- attention ----------------
        qnT = sb.tile([128, H, S], F32, tag="qnT")
        knT = sb.tile([128, H, S], F32, tag="knT")
        for si, (srcT, dstT) in enumerate(((qn, qnT), (kn, knT))):
            for g in range(2):
                base = 512 * ((2 * si + g) % 2)
                for j in range(4):
                    nc.tensor.matmul(
                        out=PS[:, base + j * S:base + (j + 1) * S],
                        lhsT=srcT[:, 4 * g + j, :],
                        rhs=ident_t[0:S, 0:S], start=True, stop=True)
                nc.vector.tensor_copy(
                    out=dstT[:, 4 * g:4 * g + 4, :],
                    in_=PS[:, base:base + 4 * S].rearrange(
                        "p (a b) -> p a b", a=4))

        att_ps = PS[0:S, 2560:3584].rearrange("s (h c) -> s h c", h=H)
        for h in range(H):
            nc.tensor.matmul(out=att_ps[:, h, 0:S], lhsT=qnT[:, h, :],
                             rhs=knT[:, h, :], start=True, stop=True)
        t0 = sb.tile([S, H, S], F32, tag="t0")
        nc.vector.tensor_tensor(
            out=t0[:, :, :], in0=att_ps[:, :, 0:S],
            in1=cmask_t[:, :].rearrange("s (a t) -> s a t", a=1)
                .to_broadcast([S, H, S]), op=ALU.add)
        m = sb.tile([S, H], F32, tag="rowmax")
        nc.vector.tensor_reduce(out=m[:, :], in_=t0[:, :, :], axis=AX.X,
                                op=ALU.max)
        mneg = sb.tile([S, H], F32, tag="mneg")
        nc.vector.tensor_scalar_mul(mneg[:, :], m[:, :], -SCALE)
        p_sb = sb.tile([S, H, S], F32, tag="p_sb")
        den = sb.tile([S, H], F32, tag="den")
        for h in range(H):
            nc.scalar.activation(out=p_sb[:, h, :], in_=t0[:, h, :],
                                 func=AF.Exp, bias=mneg[:, h:h + 1],
                                 scale=SCALE, accum_out=den[:, h:h + 1])
        pT = sb.tile([S, H, S], F32, tag="pT")
        for g in range(2):
            base = 512 * (g % 2)
            for j in range(4):
                nc.tensor.matmul(
                    out=PS[0:S, base + j * S:base + (j + 1) * S],
                    lhsT=p_sb[:, 4 * g + j, :],
                    rhs=ident_t[0:S, 0:S], start=True, stop=True)
            nc.scalar.copy(
                out=pT[:, 4 * g:4 * g + 4, :],
                in_=PS[0:S, base:base + 4 * S].rearrange(
                    "p (a b) -> p a b", a=4))

        v_sb = sb.tile([S, H, C], F32, tag="v_sb")
        nc.scalar.copy(out=v_sb[:, :, :], in_=v_all[:, :, :])
        # absorb the DVE tick of the t0 read (WAR release of the att
        # region), then the late ACT tick of the pT copies; both write
        # the same column so WAW chains them in program order
        nc.tensor.matmul(out=PS[0:S, 2560:2561], lhsT=t0[:, 0, :],
                         rhs=ident_t[0:S, 0:1], start=True, stop=True)
        nc.tensor.matmul(out=PS[0:S, 2560:2561], lhsT=pT[:, 7, :],
                         rhs=ident_t[0:S, 0:1], start=True, stop=True)
        y_ps = PS[0:S, 2560:3584].rearrange("s (h c) -> s h c", h=H)
        for h in range(H):
            nc.tensor.matmul(out=y_ps[:, h, :], lhsT=pT[:, h, :],
                             rhs=v_sb[:, h, :], start=True, stop=True)

        rden = sb.tile([S, H], F32, tag="rden")
        nc.vector.reciprocal(out=rden[:, :], in_=den[:, :])
        sigg = sb.tile([NSEL, H, C], F32, tag="sigg")
        nc.scalar.activation(out=sigg[:, :, :], in_=g_all[:, :, :],
                             func=AF.Sigmoid)
        yg = sb.tile([NSEL, H, C], F32, tag="yg")
        nc.vector.tensor_tensor(
            out=yg[:, :, :], in0=y_ps[0:NSEL, :, :],
            in1=rden[0:NSEL, :].rearrange("s (h a) -> s h a", a=1)
                .to_broadcast([NSEL, H, C]), op=ALU.mult)
        nc.vector.tensor_tensor(out=yg[:, :, :], in0=yg[:, :, :],
                                in1=sigg[:, :, :], op=ALU.mult)

        if LEVEL == 5:
            nc.sync.dma_start(out=out[:, :], in_=yg[:, 0, :])
            return
        # ---------------- output projection ----------------
        ygT = sb.tile([128, H, NSEL], F32, tag="ygT")
        nc.vector.tensor_copy(out=touch[0:S, 14:15], in_=pT[:, 7, 0:1])
        for g in range(2):
            base = 512 * (g % 2)
            for j in range(4):
                nc.tensor.matmul(
                    out=PS[:, base + j * NSEL:base + (j + 1) * NSEL],
                    lhsT=yg[:, 4 * g + j, :],
                    rhs=ident_t[0:NSEL, 0:NSEL], start=True, stop=True)
            nc.vector.tensor_copy(
                out=ygT[:, 4 * g:4 * g + 4, :],
                in_=PS[:, base:base + 4 * NSEL].rearrange(
                    "p (a b) -> p a b", a=4))

        out_ps = PS[0:NSEL, 2048:2176]
        for h in range(H):
            nc.tensor.matmul(out=out_ps, lhsT=ygT[:, h, :],
                             rhs=woT[:, h, :], start=(h == 0),
                             stop=(h == H - 1))
        out_sb = sb.tile([NSEL, C], F32, tag="out_sb")
        nc.scalar.copy(out=out_sb[:, :], in_=out_ps)
        nc.sync.dma_start(out=out[:, :], in_=out_sb[:, :])


def make_host_constants():
    ident = np.eye(128, dtype=np.float32)
    off16 = (np.arange(NSEL, dtype=np.float32) % T0).reshape(NSEL, 1)
    negio = (float(NP) - np.arange(NP, dtype=np.float32)).reshape(1, NP)
    # partition p < 64 holds sequence position p+1; partition 64 is the
    # sink (position 0)
    pos = np.where(np.arange(S) < NSEL, np.arange(S) + 1, 0)
    cmask = np.where(pos[None, :] <= pos[:, None], 0.0,
                     NEG_BIG).astype(np.float32)
    m_idx = np.arange(NSEL)
    repmat = (16.0 * (m_idx[None, :] // 16 ==
                      np.arange(4)[:, None])).astype(np.float32)
    return ident, off16, negio, cmask, repmat


_CACHE = {}


def get_nc():
    if "nc" not in _CACHE:
        nc = bacc.Bacc("TRN2", target_bir_lowering=False, debug=False,
                       num_devices=B)
        build_kernel(nc)
        nc.compile()
        _CACHE["nc"] = nc
    return _CACHE["nc"]


def make_in_maps(inputs):
    x = np.ascontiguousarray(inputs["x"], dtype=np.float32)
    cos = np.ascontiguousarray(np.asarray(inputs["cos"]).reshape(S, 64),
                               dtype=np.float32)
    sin = np.ascontiguousarray(np.asarray(inputs["sin"]).reshape(S, 64),
                               dtype=np.float32)
    sinkv = np.ascontiguousarray(np.asarray(inputs["sink"]).reshape(H, C),
                                 dtype=np.float32)
    wqkvg = np.ascontiguousarray(inputs["W_qkvg"], dtype=np.float32)
    pw = np.ascontiguousarray(inputs["patch_w"], dtype=np.float32)
    wout = np.ascontiguousarray(inputs["W_out"], dtype=np.float32)
    tao = np.ascontiguousarray(
        np.broadcast_to(np.asarray(inputs["tao"], dtype=np.float32), (S, 2)))
    ident, off16, negio, cmask, repmat = make_host_constants()
    in_maps = []
    for b in range(B):
        in_maps.append({
            "xb": np.ascontiguousarray(x[b]),
            "pw": pw, "wqkvg": wqkvg, "wout": wout, "sink": sinkv,
            "cosd": cos, "sind": sin, "tao": tao, "ident": ident,
            "off16": off16, "negio": negio, "cmask": cmask,
            "repmat": repmat,
        })
    return in_maps


def kernel(**inputs):
    nc = get_nc()
    in_maps = make_in_maps(inputs)
    res = run_bass_kernel_spmd(nc, in_maps, core_ids=list(range(B)))
    return np.stack([r["out"] for r in res.results], axis=0)


if __name__ == "__main__":
    nc = get_nc()
    print("build ok:", len(nc.m.functions[0].allocations), "allocations")



# revision 25
# speedup vs baseline: 2.0045x; 2.0045x over previous
"""Trainium2 Bass kernel for nn_AttentionOnDetail (sparse patch attention).

Data-parallel over batch B=8 across 8 NeuronCores; one batch per core.
Per core:
  phase 1: stream x[b] (4MB) in patch-major tiles [128 patches, 2048];
           per-patch sum-of-squares (ACT Square+accum) and dot with
           patch_w (DVE stt+accum) -> 512 logits.
  top-4:   top-8 values -> 4th-value threshold -> mask * (512-i) ->
           max_index -> 4 selected patch ids ascending; expand to 64
           token ids with one STT; indirect-DMA gather of x_sel.
  phase 2: qkvg projection of only the 64 selected tokens in bf16
           against a host-pretransposed W_qkvg.T; one strided
           SBUF->SBUF DMA permutes token-major qkvg into positional
           q/k/v/g [65,4,8,128] (sink appended); RoPE + rmsnorm (stats
           taken pre-rope: rotations preserve norms) + tao folded into
           the softmax exp scale; causal attention via multiplicative
           bf16 mask, no max-subtraction (|logits| <= ~16 so exp is
           safe in fp32 range); sigmoid gating; output projection
           against host-pretransposed W_out.
"""

import sys
import numpy as np
import ml_dtypes

for _p in ("/opt/trn_rl_repo",):
    if _p not in sys.path:
        sys.path.insert(0, _p)

import concourse.bass as bass
import concourse.bacc as bacc
import concourse.tile as tile
from concourse import mybir
from concourse.bass_utils import run_bass_kernel_spmd

F32 = mybir.dt.float32
BF16 = mybir.dt.bfloat16
I32 = mybir.dt.int32
U32 = mybir.dt.uint32
AF = mybir.ActivationFunctionType
ALU = mybir.AluOpType
AX = mybir.AxisListType

B, T, C, H, T0 = 8, 8192, 128, 8, 16
NP = T // T0          # 512 patches
PATCH = T0 * C        # 2048 elements per patch
S = 65                # sink + 64 selected tokens
NSEL = 64
FQ = 4 * C * H        # 4096
HC = H * C            # 1024
EPS = 1.1920929e-07
SCALE = 1.0 / float(np.sqrt(np.float32(C)))
BF = np.dtype(ml_dtypes.bfloat16)


def rap(t, apl, offset=0):
    """Raw AP over a tile/AP's storage, flat element strides."""
    base = t if isinstance(t, bass.AP) else t[:]
    return bass.AP(tensor=base.tensor, offset=base.offset + offset,
                   ap=[list(x) for x in apl])


def build_kernel(nc):
    xb = nc.dram_tensor("xb", [T, C], F32, kind="ExternalInput")
    pw = nc.dram_tensor("pw", [1, PATCH], F32, kind="ExternalInput")
    wqt = nc.dram_tensor("wqt", [C, FQ], BF16, kind="ExternalInput")
    wot = nc.dram_tensor("wot", [C, HC], BF16, kind="ExternalInput")
    sinkr = nc.dram_tensor("sinkr", [1, 3 * HC], BF16, kind="ExternalInput")
    cosb = nc.dram_tensor("cosb", [S, 64], BF16, kind="ExternalInput")
    sindb = nc.dram_tensor("sindb", [S, 2, 64], BF16, kind="ExternalInput")
    cm01 = nc.dram_tensor("cm01", [S, S], F32, kind="ExternalInput")
    negio = nc.dram_tensor("negio", [1, NP], F32, kind="ExternalInput")
    off16r = nc.dram_tensor("off16r", [1, NSEL], F32, kind="ExternalInput")
    tsc = nc.dram_tensor("tsc", [S, 1], F32, kind="ExternalInput")
    identf = nc.dram_tensor("identf", [C, C], F32, kind="ExternalInput")
    identb = nc.dram_tensor("identb", [S, S], BF16, kind="ExternalInput")
    out = nc.dram_tensor("out", [NSEL, C], F32, kind="ExternalOutput")

    with tile.TileContext(nc) as tc:
        _emit(tc, nc, xb, pw, wqt, wot, sinkr, cosb, sindb, cm01, negio,
              off16r, tsc, identf, identb, out)
    return nc


def _emit(tc, nc, xb, pw, wqt, wot, sinkr, cosb, sindb, cm01, negio,
          off16r, tsc, identf, identb, out):
    import os
    LEVEL = int(os.environ.get("KLEVEL", "9"))
    from contextlib import ExitStack
    ctx = ExitStack()
    with ctx:
        const1 = ctx.enter_context(tc.tile_pool(name="const1", bufs=1))
        xpool = ctx.enter_context(tc.tile_pool(name="xpool", bufs=3))
        junkp = ctx.enter_context(tc.tile_pool(name="junkp", bufs=1))
        stat = ctx.enter_context(tc.tile_pool(name="stat", bufs=1))
        sb = ctx.enter_context(tc.tile_pool(name="sb", bufs=1))
        psall = ctx.enter_context(tc.tile_pool(name="psall", bufs=1,
                                               space="PSUM"))
        dramp = ctx.enter_context(tc.tile_pool(name="dramp", bufs=1,
                                               space="DRAM"))
        # one tile owning all 8 PSUM banks; regions choreographed manually
        PS = psall.tile([128, 4096], F32)
        # bank/col layout (f32 cols):
        #   3584:3712  logits transpose [4,128]
        #   3712:3776  x_selT [128,64]
        #   0:4096     qkvg mm [64, 512k]
        #   0:1024     qnT  (h*128)         [after qkvg drained]
        #   1024:2048  knT  (h*128)
        #   2048:3072  att  (h*128), later out mm [64, 2048:2176]
        #   3072:4096  pT   (h*128), later ygT [128, 3072+h*64]
        #   0:1024     y    (h*128)         [after qnT consumed]

        # ---------------- constant / weight DMAs ----------------
        # everything rides the SP queue: pw+identf (needed early) before
        # the x tiles, weights + late tables after. The Pool queue stays
        # free so partition_broadcast and the gather run immediately.
        pw_sb = const1.tile([1, PATCH], F32)
        nc.sync.dma_start(out=pw_sb[:, :], in_=pw[:, :])
        identf_t = const1.tile([C, C], F32)
        xts = []
        for i in range(4):
            xp = xpool.tile([128, PATCH], F32, tag="xp")
            nc.sync.dma_start(
                out=xp[:, :],
                in_=rap(xb[:, :], [[PATCH, 128], [1, PATCH]],
                        offset=i * 128 * PATCH))
            xts.append(xp)
            if i == 0:
                nc.sync.dma_start(out=identf_t[:, :], in_=identf[:, :])
        negio_t = const1.tile([1, NP], F32)
        nc.sync.dma_start(out=negio_t[:, :], in_=negio[:, :])
        off16_t = const1.tile([1, NSEL], F32)
        nc.sync.dma_start(out=off16_t[:, :], in_=off16r[:, :])
        wqt_sb = const1.tile([C, FQ], BF16)
        nc.sync.dma_start(out=wqt_sb[:, :], in_=wqt[:, :])
        wot_sb = const1.tile([C, H, C], BF16)
        nc.sync.dma_start(
            out=wot_sb[:, :, :],
            in_=wot[:, :].rearrange("c (h d) -> c h d", h=H))
        identb_t = const1.tile([S, S], BF16)
        nc.sync.dma_start(out=identb_t[:, :], in_=identb[:, :])
        cos_t = const1.tile([S, 64], BF16)
        nc.sync.dma_start(out=cos_t[:, :], in_=cosb[:, :])
        sinD = const1.tile([S, 2, 64], BF16)
        nc.sync.dma_start(out=sinD[:, :, :], in_=sindb[:, :, :])
        cm_t = const1.tile([S, S], F32)
        nc.sync.dma_start(out=cm_t[:, :], in_=cm01[:, :])
        tsc_t = const1.tile([S, 1], F32)
        nc.sync.dma_start(out=tsc_t[:, :], in_=tsc[:, :])
        sink_t = const1.tile([1, 3 * HC], BF16)
        nc.sync.dma_start(out=sink_t[:, :], in_=sinkr[:, :])

        eps_t = const1.tile([128, 1], F32)
        nc.vector.memset(eps_t[:, :], EPS)
        ones1_t = const1.tile([1, 1], F32)
        nc.vector.memset(ones1_t[:, :], 1.0)

        # ---------------- phase 1: per-patch stats ----------------
        # preload act func set 6 (ln+exp+square+copy) once; the bacc
        # table pass sees it and inserts no further loads
        nc.scalar.add_instruction(mybir.InstLoadActFuncSet(
            name=f"I-{nc.next_id()}", engine=mybir.EngineType.Activation,
            act_func_set_id=6, ins=[], outs=[]))
        # pw broadcast to 128 partitions on the Pool engine
        pwB = const1.tile([128, PATCH], F32)
        nc.gpsimd.partition_broadcast(pwB[:, :], pw_sb[:, :])

        # per-tile pipeline: square+accum (ACT) || dot (DVE) ->
        # rsqrt via ln/exp (ACT) -> logit col (DVE) -> PE transpose ->
        # row chunk copy (ACT); the [1,512] row assembles in SBUF with
        # no SBUF->SBUF DMA.
        ssc = stat.tile([128, 4], F32)
        dotc = stat.tile([128, 4], F32)
        lnc = stat.tile([128, 4], F32)
        rsqc = stat.tile([128, 4], F32)
        lcol = stat.tile([128, 4], F32)
        logits_row = stat.tile([1, NP], F32)
        for i in range(4):
            xp = xts[i]
            junk = junkp.tile([128, PATCH], F32, tag="junk")
            nc.scalar.activation(out=junk[:, :], in_=xp[:, :], func=AF.Square,
                                 accum_out=ssc[:, i:i + 1])
            junk2 = junkp.tile([128, PATCH], F32, tag="junk2")
            nc.vector.scalar_tensor_tensor(
                out=junk2[:, :], in0=xp[:, :], scalar=1.0, in1=pwB[:, :],
                op0=ALU.mult, op1=ALU.mult, accum_out=dotc[:, i:i + 1])
            nc.scalar.activation(out=lnc[:, i:i + 1], in_=ssc[:, i:i + 1],
                                 func=AF.Ln, bias=eps_t[:, 0:1],
                                 scale=1.0 / PATCH)
            nc.scalar.activation(out=rsqc[:, i:i + 1], in_=lnc[:, i:i + 1],
                                 func=AF.Exp, scale=-0.5)
            nc.vector.tensor_mul(lcol[:, i:i + 1], dotc[:, i:i + 1],
                                 rsqc[:, i:i + 1])
            nc.tensor.transpose(out=PS[0:1, 3584 + i * 128:3712 + i * 128],
                                in_=lcol[:, i:i + 1],
                                identity=identf_t[:, :])
        # transpose outputs land contiguous in PSUM: one row copy
        nc.scalar.copy(out=logits_row[:, :], in_=PS[0:1, 3584:4096])

        if LEVEL == 1:
            nc.sync.dma_start(out=out[:, 0:4], in_=lcol[0:64, :])
            return
        # ---------------- top-4 selection ----------------
        max8 = stat.tile([1, 8], F32)
        nc.vector.max(out=max8[:, :], in_=logits_row[:, :])
        idx8 = stat.tile([1, 8], U32)
        nc.vector.max_index(out=idx8[:, :], in_max=max8[:, :],
                            in_values=logits_row[:, :])
        # sort the top-4 patch ids ascending: negate, pad low, desc max
        # sort => negated ids ascending; no negative-stride reads
        idxp = stat.tile([1, 8], F32)
        nc.vector.memset(idxp[:, :], -1.0e9)
        idxf = stat.tile([1, 8], F32)
        nc.vector.tensor_copy(out=idxf[:, :], in_=idx8[:, :])
        nc.vector.tensor_scalar_mul(idxp[:, 0:4], idxf[:, 0:4], -1.0)
        idxs = stat.tile([1, 8], F32)
        nc.vector.max(out=idxs[:, :], in_=idxp[:, :])
        # token ids: tok[16*a + j] = 16*id[a] + j in one STT
        tok_f = stat.tile([1, NSEL], F32)
        nc.vector.scalar_tensor_tensor(
            out=tok_f[:, :],
            in0=rap(idxs, [[8, 1], [1, 4], [0, 16]]),
            scalar=-16.0, in1=off16_t[:, :].rearrange("p (a b) -> p a b", a=4),
            op0=ALU.mult, op1=ALU.add)
        # gather wants a column of offsets: transpose via K=1 matmul
        nc.tensor.matmul(out=PS[0:NSEL, 3776:3777], lhsT=tok_f[:, :],
                         rhs=ones1_t[:, :], start=True, stop=True)
        tok_i = stat.tile([NSEL, 1], I32)
        nc.vector.tensor_copy(out=tok_i[:, :], in_=PS[0:NSEL, 3776:3777])

        x_sel = sb.tile([NSEL, C], F32, tag="x_sel")
        nc.gpsimd.indirect_dma_start(
            out=x_sel[:, :], out_offset=None, in_=xb[:, :],
            in_offset=bass.IndirectOffsetOnAxis(ap=tok_i[:, 0:1], axis=0))

        if LEVEL == 2:
            nc.sync.dma_start(out=out[:, :], in_=x_sel[:, :])
            return
        # ---------------- qkvg projection (64 tokens, bf16) ----------------
        # warm the PE pstate behind the gather/copy so qkvg runs warm
        nc.tensor.transpose(out=PS[0:128, 3712:3776], in_=x_sel[:, :],
                            identity=identf_t[0:NSEL, 0:NSEL])
        x_selT = sb.tile([128, NSEL], BF16, tag="x_selT")
        nc.scalar.copy(out=x_selT[:, :], in_=PS[0:128, 3712:3776])
        for w in range(8):
            nc.tensor.matmul(out=PS[0:1, 3583:3584], lhsT=x_sel[:, 0:1],
                             rhs=identf_t[0:NSEL, 0:1], start=True, stop=True)

        qkvg_sb = sb.tile([NSEL, FQ], BF16, tag="qkvg")
        for k in range(8):
            nc.tensor.matmul(
                out=PS[0:NSEL, 512 * k:512 * (k + 1)],
                lhsT=x_selT[:, :],
                rhs=wqt_sb[:, 512 * k:512 * (k + 1)],
                start=True, stop=True)
            if k % 2 == 0:
                nc.scalar.copy(out=qkvg_sb[:, 512 * k:512 * (k + 1)],
                               in_=PS[0:NSEL, 512 * k:512 * (k + 1)])
            else:
                nc.vector.tensor_copy(
                    out=qkvg_sb[:, 512 * k:512 * (k + 1)],
                    in_=PS[0:NSEL, 512 * k:512 * (k + 1)])

        # permute via DRAM bounce (SBUF DMA APs cross partitions only in
        # dim0): one flat SBUF->DRAM copy, then per-tensor permuted
        # DRAM->SBUF reads. dst partition s = 16a+4e+b reads src row
        # 16a+4X+e at feature block b*1024 -- on the flat DRAM side this
        # collapses to 3 dims [a][e][4096 contiguous].
        qflat = dramp.tile([NSEL, FQ], BF16)
        nc.sync.dma_start(out=qflat[:, :], in_=qkvg_sb[:, :])
        qkvg_all = sb.tile([S, 4, H, C], BF16, tag="qkvg_all")
        for X, eng in ((0, nc.sync), (1, nc.gpsimd), (2, nc.scalar),
                       (3, nc.sync)):
            eng.dma_start(
                out=rap(qkvg_all, [[4 * HC, NSEL], [1, HC]], offset=X * HC),
                in_=rap(qflat[:, :], [[FQ * 16, 4], [FQ, 4], [1, FQ]],
                        offset=X * 4 * FQ))
        # sink row for q/k/v at position partition 64
        nc.scalar.dma_start(out=rap(qkvg_all, [[4 * HC, 1], [1, 3 * HC]],
                                    offset=NSEL * 4 * HC),
                            in_=sink_t[:, :])

        if LEVEL == 3:
            dbg = sb.tile([NSEL, C], F32, tag="dbg3")
            nc.scalar.copy(out=dbg[:, :], in_=qkvg_all[0:NSEL, 0, 0, :])
            nc.sync.dma_start(out=out[:, :], in_=dbg[:, :])
            return
        # ---------------- RoPE + rmsnorm (stats pre-rope) ----------------
        qk = qkvg_all[:, 0:2, :, :]  # [65, 2, 8, 128] bf16
        # sum of squares per (s, X, h) -- rope is a rotation, norms equal
        junk3 = junkp.tile([S, 2, H, C], BF16, tag="junk3")
        nc.scalar.activation(out=junk3[:, :, :, :], in_=qk, func=AF.Square)
        ms = stat.tile([S, 2, H], F32)
        nc.vector.tensor_reduce(out=ms[:, :, :], in_=junk3[:, :, :, :],
                                axis=AX.X, op=ALU.add)
        # rsqrt via ln/exp (same ACT table set as softmax exp)
        lnm = stat.tile([S, 2, H], F32)
        nc.scalar.activation(out=lnm[:, :, :], in_=ms[:, :, :], func=AF.Ln,
                             bias=eps_t[0:S, 0:1], scale=1.0 / C)
        rsq = stat.tile([S, 2, H], BF16)
        nc.scalar.activation(out=rsq[:, :, :], in_=lnm[:, :, :], func=AF.Exp,
                             scale=-0.5)

        # rope: r = qk * cos_full + swap(qk) * [sin, -sin]
        # (APs hand-collapsed to <=3 free dims for the TensorTensor ISA)
        r1 = sb.tile([S, 2, H, 2, 64], BF16, tag="rope_r1")
        r1f = rap(r1, [[2048, S], [1, 2048]])
        qk_flat = rap(qkvg_all, [[4 * HC, S], [1, 2048]])
        cos_b = rap(cos_t, [[64, S], [0, 16], [1, 64]])
        cos_b2 = rap(cos_t, [[64, S], [0, 32], [1, 64]])
        nc.vector.tensor_tensor(
            out=rap(r1, [[2048, S], [64, 32], [1, 64]]),
            in0=rap(qkvg_all, [[4 * HC, S], [64, 32], [1, 64]]),
            in1=cos_b2, op=ALU.mult)
        r2 = sb.tile([S, 2, H, 2, 64], BF16, tag="rope_r2")
        # swap(qk): k=0 reads hi half (offset 64), k=1 reads lo half
        qk_swap = rap(qkvg_all, [[4 * HC, S], [128, 16], [-64, 2], [1, 64]],
                      offset=64)
        sin_b = rap(sinD, [[128, S], [0, 16], [64, 2], [1, 64]])
        nc.vector.tensor_tensor(
            out=rap(r2, [[2048, S], [128, 16], [64, 2], [1, 64]]),
            in0=qk_swap, in1=sin_b, op=ALU.mult)
        nc.vector.tensor_add(out=r1f, in0=r1f,
                             in1=rap(r2, [[2048, S], [1, 2048]]))
        # scale by rsq -> normalized q/k (tao folded into softmax scale)
        qkn = sb.tile([S, 2, H, C], BF16, tag="qkn")
        nc.vector.tensor_tensor(
            out=rap(qkn, [[2048, S], [1, 2048]]),
            in0=r1f,
            in1=rap(rsq, [[16, S], [1, 16], [0, 128]]),
            op=ALU.mult)

        if LEVEL == 4:
            dbg = sb.tile([NSEL, C], F32, tag="dbg4")
            nc.scalar.copy(out=dbg[:, :], in_=qkn[0:NSEL, 0, 0, :])
            nc.sync.dma_start(out=out[:, :], in_=dbg[:, :])
            return
        # ---------------- attention ----------------
        # transposes: qnT/knT [128, 8, 65] bf16 (PSUM viewed as bf16)
        PSB = PS[:, :].bitcast(BF16)  # [[8192,128],[1,8192]] bf16 units
        qnT = sb.tile([128, H, S], BF16, tag="qnT")
        knT = sb.tile([128, H, S], BF16, tag="knT")
        for xi, dstT in ((0, qnT), (1, knT)):
            base = 2048 * xi
            for h in range(H):
                nc.tensor.transpose(
                    out=rap(PSB, [[8192, 128], [1, S]],
                            offset=base + h * 256),
                    in_=qkn[:, xi, h, :], identity=identb_t[:, :])
            if xi == 0:
                nc.scalar.copy(
                    out=dstT[:, :, :],
                    in_=rap(PSB, [[8192, 128], [256, H], [1, S]],
                            offset=base))
            else:
                nc.vector.tensor_copy(
                    out=dstT[:, :, :],
                    in_=rap(PSB, [[8192, 128], [256, H], [1, S]],
                            offset=base))

        # preload -BIG causal bias into the att PSUM region; matmuls
        # accumulate on top (start=False), so exp gives masked p directly
        att_v = PS[0:S, 2048:3072].rearrange("s (h c) -> s h c", h=H)[:, :, 0:S]
        nc.vector.tensor_copy(
            out=att_v,
            in_=cm_t[:, :].rearrange("s (a t) -> s a t", a=1)
                .to_broadcast([S, H, S]))
        for h in range(H):
            nc.tensor.matmul(out=PS[0:S, 2048 + h * 128:2048 + h * 128 + S],
                             lhsT=qnT[:, h, :], rhs=knT[:, h, :],
                             start=False, stop=True, skip_group_check=True)
        # p = exp((att + bias) * tao0*tao1*SCALE)   (no max-sub needed)
        pm = sb.tile([S, H, S], BF16, tag="pm")
        nc.scalar.activation(out=pm[:, :, :], in_=att_v, func=AF.Exp,
                             scale=tsc_t[:, 0:1])
        den = stat.tile([S, H], F32)
        nc.vector.tensor_reduce(out=den[:, :], in_=pm[:, :, :], axis=AX.X,
                                op=ALU.add)
        rden = stat.tile([S, H], F32)
        nc.vector.reciprocal(out=rden[:, :], in_=den[:, :])

        for h in range(H):
            nc.tensor.transpose(
                out=rap(PSB, [[8192, S], [1, S]], offset=6144 + h * 256),
                in_=pm[:, h, :], identity=identb_t[:, :])
        pT = sb.tile([S, H, S], BF16, tag="pT")
        nc.scalar.copy(
            out=pT[:, :, :],
            in_=rap(PSB, [[8192, S], [256, H], [1, S]], offset=6144))

        v_b = sb.tile([S, H, C], BF16, tag="v_b")
        nc.scalar.copy(out=v_b[:, :, :], in_=qkvg_all[:, 2, :, :])
        for h in range(H):
            nc.tensor.matmul(out=PS[0:S, h * 128:(h + 1) * 128],
                             lhsT=pT[:, h, :], rhs=v_b[:, h, :],
                             start=True, stop=True)

        # ---------------- gating ----------------
        # sigmoid(g) = 1/(1+exp(-g)) -- stays in the ln/exp table set
        eg = sb.tile([NSEL, H, C], BF16, tag="eg")
        nc.scalar.activation(out=eg[:, :, :], in_=qkvg_all[0:NSEL, 3, :, :],
                             func=AF.Exp, scale=-1.0)
        eg1 = sb.tile([NSEL, H, C], F32, tag="eg1")
        nc.gpsimd.tensor_scalar_add(eg1[:, :, :], eg[:, :, :], 1.0)
        sig = sb.tile([NSEL, H, C], F32, tag="sig")
        nc.vector.reciprocal(out=sig[:, :, :], in_=eg1[:, :, :])
        # fold 1/den into the gate so y is scaled once
        sig2 = sb.tile([NSEL, H, C], F32, tag="sig2")
        nc.vector.tensor_tensor(
            out=sig2[:, :, :], in0=sig[:, :, :],
            in1=rden[0:NSEL, :].rearrange("s (h a) -> s h a", a=1)
                .to_broadcast([NSEL, H, C]), op=ALU.mult)
        yg = sb.tile([NSEL, H, C], BF16, tag="yg")
        nc.vector.tensor_tensor(
            out=yg[:, :, :],
            in0=PS[0:NSEL, 0:1024].rearrange("s (h c) -> s h c", h=H),
            in1=sig2[:, :, :], op=ALU.mult)

        if LEVEL == 5:
            dbg = sb.tile([NSEL, C], F32, tag="dbg5")
            nc.scalar.copy(out=dbg[:, :], in_=yg[:, 0, :])
            nc.sync.dma_start(out=out[:, :], in_=dbg[:, :])
            return
        # ---------------- output projection ----------------
        for h in range(H):
            nc.tensor.transpose(
                out=rap(PSB, [[8192, 128], [1, NSEL]], offset=6144 + h * 128),
                in_=yg[:, h, :], identity=identb_t[0:NSEL, 0:NSEL])
        ygT = sb.tile([128, H, NSEL], BF16, tag="ygT")
        nc.scalar.copy(
            out=ygT[:, :, :],
            in_=rap(PSB, [[8192, 128], [128, H], [1, NSEL]], offset=6144))
        for h in range(H):
            nc.tensor.matmul(out=PS[0:NSEL, 2048:2176], lhsT=ygT[:, h, :],
                             rhs=wot_sb[:, h, :], start=(h == 0),
                             stop=(h == H - 1))
        out_sb = sb.tile([NSEL, C], F32, tag="out_sb")
        nc.scalar.copy(out=out_sb[:, :], in_=PS[0:NSEL, 2048:2176])
        nc.sync.dma_start(out=out[:, :], in_=out_sb[:, :])


def make_host_constants():
    identf = np.eye(C, dtype=np.float32)
    identb = np.eye(S, dtype=BF)
    off16r = (np.arange(NSEL, dtype=np.float32) % T0).reshape(1, NSEL)
    negio = (float(NP) - np.arange(NP, dtype=np.float32)).reshape(1, NP)
    # partition p < 64 holds sequence position p+1; partition 64 is the
    # sink (position 0); multiplicative causal mask
    pos = np.where(np.arange(S) < NSEL, np.arange(S) + 1, 0)
    cm01 = np.where(pos[None, :] <= pos[:, None], 0.0, -1e30).astype(np.float32)
    return identf, identb, off16r, negio, cm01


_CACHE = {}


def get_nc():
    if "nc" not in _CACHE:
        nc = bacc.Bacc("TRN2", target_bir_lowering=False, debug=False,
                       num_devices=B)
        build_kernel(nc)
        nc.compile()
        _CACHE["nc"] = nc
    return _CACHE["nc"]


def make_in_maps(inputs):
    x = np.ascontiguousarray(inputs["x"], dtype=np.float32)
    cos = np.asarray(inputs["cos"]).reshape(S, 64).astype(np.float32)
    sin = np.asarray(inputs["sin"]).reshape(S, 64).astype(np.float32)
    # permute rows: partition p = seq position p+1 (p<64), partition 64 = pos 0
    perm = np.concatenate([np.arange(1, S), [0]])
    cosb = np.ascontiguousarray(cos[perm]).astype(BF)
    sindb = np.ascontiguousarray(
        np.stack([sin[perm], -sin[perm]], axis=1)).astype(BF)
    sinkv = np.asarray(inputs["sink"]).reshape(1, HC).astype(np.float32)
    sinkr = np.ascontiguousarray(
        np.broadcast_to(sinkv, (3, HC)).reshape(1, 3 * HC)).astype(BF)
    wqkvg = np.asarray(inputs["W_qkvg"], dtype=np.float32)
    wqt = np.ascontiguousarray(wqkvg.T).astype(BF)                # [128, 4096]
    wout = np.asarray(inputs["W_out"], dtype=np.float32)          # [128, 1024]
    # wot[c, h, co] = W_out[co, h*128+c]
    wot = np.ascontiguousarray(
        wout.T.reshape(H, C, C).transpose(1, 0, 2).reshape(C, HC)).astype(BF)
    pwv = np.asarray(inputs["patch_w"], dtype=np.float32).reshape(1, PATCH)
    tao = np.asarray(inputs["tao"], dtype=np.float32)
    tscv = np.full((S, 1), float(tao[0]) * float(tao[1]) * SCALE,
                   dtype=np.float32)
    identf, identb, off16r, negio, cm01 = make_host_constants()
    in_maps = []
    for b in range(B):
        in_maps.append({
            "xb": np.ascontiguousarray(x[b]),
            "pw": pwv, "wqt": wqt, "wot": wot, "sinkr": sinkr,
            "cosb": cosb, "sindb": sindb, "cm01": cm01, "negio": negio,
            "off16r": off16r, "tsc": tscv, "identf": identf,
            "identb": identb,
        })
    return in_maps


def kernel(**inputs):
    nc = get_nc()
    in_maps = make_in_maps(inputs)
    res = run_bass_kernel_spmd(nc, in_maps, core_ids=list(range(B)))
    return np.stack([r["out"] for r in res.results], axis=0)


if __name__ == "__main__":
    nc = get_nc()
    print("build ok:", len(nc.m.functions[0].allocations), "allocations")


# revision 44
# speedup vs baseline: 2.1400x; 1.0676x over previous
"""Trainium2 Bass kernel for nn_AttentionOnDetail (sparse patch attention).

Data-parallel over batch B=8 across 8 NeuronCores; one batch per core.
Per core:
  phase 1: stream x[b] (4MB) in patch-major tiles [128 patches, 2048];
           per-patch sum-of-squares (ACT Square+accum) and dot with
           patch_w (DVE stt+accum) -> 512 logits.
  top-4:   top-8 values -> 4th-value threshold -> mask * (512-i) ->
           max_index -> 4 selected patch ids ascending; expand to 64
           token ids with one STT; indirect-DMA gather of x_sel.
  phase 2: qkvg projection of only the 64 selected tokens in bf16
           against a host-pretransposed W_qkvg.T; one strided
           SBUF->SBUF DMA permutes token-major qkvg into positional
           q/k/v/g [65,4,8,128] (sink appended); RoPE + rmsnorm (stats
           taken pre-rope: rotations preserve norms) + tao folded into
           the softmax exp scale; causal attention via multiplicative
           bf16 mask, no max-subtraction (|logits| <= ~16 so exp is
           safe in fp32 range); sigmoid gating; output projection
           against host-pretransposed W_out.
"""

import sys
import numpy as np
import ml_dtypes

for _p in ("/opt/trn_rl_repo",):
    if _p not in sys.path:
        sys.path.insert(0, _p)

import concourse.bass as bass
import concourse.bacc as bacc
import concourse.tile as tile
from concourse import mybir
from concourse.bass_utils import run_bass_kernel_spmd

F32 = mybir.dt.float32
BF16 = mybir.dt.bfloat16
I32 = mybir.dt.int32
U32 = mybir.dt.uint32
AF = mybir.ActivationFunctionType
ALU = mybir.AluOpType
AX = mybir.AxisListType

B, T, C, H, T0 = 8, 8192, 128, 8, 16
NP = T // T0          # 512 patches
PATCH = T0 * C        # 2048 elements per patch
S = 65                # sink + 64 selected tokens
NSEL = 64
FQ = 4 * C * H        # 4096
HC = H * C            # 1024
EPS = 1.1920929e-07
SCALE = 1.0 / float(np.sqrt(np.float32(C)))
BF = np.dtype(ml_dtypes.bfloat16)


def rap(t, apl, offset=0):
    """Raw AP over a tile/AP's storage, flat element strides."""
    base = t if isinstance(t, bass.AP) else t[:]
    return bass.AP(tensor=base.tensor, offset=base.offset + offset,
                   ap=[list(x) for x in apl])


def build_kernel(nc):
    xb = nc.dram_tensor("xb", [T, C], F32, kind="ExternalInput")
    pw = nc.dram_tensor("pw", [1, PATCH], F32, kind="ExternalInput")
    wqt = nc.dram_tensor("wqt", [C, FQ], BF16, kind="ExternalInput")
    wot = nc.dram_tensor("wot", [C, HC], BF16, kind="ExternalInput")
    sinkr = nc.dram_tensor("sinkr", [1, 3 * HC], BF16, kind="ExternalInput")
    cosb = nc.dram_tensor("cosb", [C, S], BF16, kind="ExternalInput")
    sindb = nc.dram_tensor("sindb", [C, S], BF16, kind="ExternalInput")
    cm01 = nc.dram_tensor("cm01", [S, S], F32, kind="ExternalInput")
    negio = nc.dram_tensor("negio", [1, NP], F32, kind="ExternalInput")
    off16r = nc.dram_tensor("off16r", [1, 16 + NSEL], F32, kind="ExternalInput")
    tsc = nc.dram_tensor("tsc", [S, 1], F32, kind="ExternalInput")
    identf = nc.dram_tensor("identf", [C, C], F32, kind="ExternalInput")
    identb = nc.dram_tensor("identb", [S, S], BF16, kind="ExternalInput")
    out = nc.dram_tensor("out", [NSEL, C], F32, kind="ExternalOutput")

    with tile.TileContext(nc) as tc:
        _emit(tc, nc, xb, pw, wqt, wot, sinkr, cosb, sindb, cm01, negio,
              off16r, tsc, identf, identb, out)
    return nc


def _emit(tc, nc, xb, pw, wqt, wot, sinkr, cosb, sindb, cm01, negio,
          off16r, tsc, identf, identb, out):
    import os
    LEVEL = int(os.environ.get("KLEVEL", "9"))
    from contextlib import ExitStack
    ctx = ExitStack()
    with ctx:
        const1 = ctx.enter_context(tc.tile_pool(name="const1", bufs=1))
        xpool = ctx.enter_context(tc.tile_pool(name="xpool", bufs=3))
        junkp = ctx.enter_context(tc.tile_pool(name="junkp", bufs=1))
        stat = ctx.enter_context(tc.tile_pool(name="stat", bufs=1))
        sb = ctx.enter_context(tc.tile_pool(name="sb", bufs=1))
        psall = ctx.enter_context(tc.tile_pool(name="psall", bufs=1,
                                               space="PSUM"))
        dramp = ctx.enter_context(tc.tile_pool(name="dramp", bufs=1,
                                               space="DRAM"))
        # one tile owning all 8 PSUM banks; regions choreographed manually
        PS = psall.tile([128, 4096], F32)
        # bank/col layout (f32 cols):
        #   3584:3712  logits transpose [4,128]
        #   3712:3776  x_selT [128,64]
        #   0:4096     qkvg mm [64, 512k]
        #   0:1024     qnT  (h*128)         [after qkvg drained]
        #   1024:2048  knT  (h*128)
        #   2048:3072  att  (h*128), later out mm [64, 2048:2176]
        #   3072:4096  pT   (h*128), later ygT [128, 3072+h*64]
        #   0:1024     y    (h*128)         [after qnT consumed]

        # ---------------- constant / weight DMAs ----------------
        # everything rides the SP queue: pw+identf (needed early) before
        # the x tiles, weights + late tables after. The Pool queue stays
        # free so partition_broadcast and the gather run immediately.
        pw_sb = const1.tile([1, PATCH], F32)
        nc.sync.dma_start(out=pw_sb[:, :], in_=pw[:, :])
        identf_t = const1.tile([C, C], F32)
        xts = []
        for i in range(4):
            xp = xpool.tile([128, PATCH], F32, tag="xp")
            if i < 3:
                nc.sync.dma_start(
                    out=xp[:, :],
                    in_=rap(xb[:, :], [[PATCH, 128], [1, PATCH]],
                            offset=i * 128 * PATCH))
            else:
                # last tile split in halves so its stats pipeline with
                # the second half's transfer
                for hf in range(2):
                    nc.sync.dma_start(
                        out=xp[:, 1024 * hf:1024 * (hf + 1)],
                        in_=rap(xb[:, :], [[PATCH, 128], [1, 1024]],
                                offset=i * 128 * PATCH + 1024 * hf))
            xts.append(xp)
            if i == 0:
                nc.sync.dma_start(out=identf_t[:, :], in_=identf[:, :])
        negio_t = const1.tile([1, NP], F32)
        nc.sync.dma_start(out=negio_t[:, :], in_=negio[:, :])
        off16_t = const1.tile([1, 16 + NSEL], F32)
        nc.sync.dma_start(out=off16_t[:, :], in_=off16r[:, :])
        wqt_sb = const1.tile([C, FQ], BF16)
        nc.sync.dma_start(out=wqt_sb[:, :], in_=wqt[:, :])
        wot_sb = const1.tile([C, H, C], BF16)
        nc.sync.dma_start(
            out=wot_sb[:, :, :],
            in_=wot[:, :].rearrange("c (h d) -> c h d", h=H))
        identb_t = const1.tile([S, S], BF16)
        nc.sync.dma_start(out=identb_t[:, :], in_=identb[:, :])
        cosT_t = const1.tile([C, S], BF16)
        nc.sync.dma_start(out=cosT_t[:, :], in_=cosb[:, :])
        sinT_t = const1.tile([C, S], BF16)
        nc.sync.dma_start(out=sinT_t[:, :], in_=sindb[:, :])
        cm_t = const1.tile([S, S], F32)
        nc.sync.dma_start(out=cm_t[:, :], in_=cm01[:, :])
        tsc_t = const1.tile([S, 1], F32)
        nc.sync.dma_start(out=tsc_t[:, :], in_=tsc[:, :])
        sink_t = const1.tile([1, 3 * HC], BF16)
        nc.sync.dma_start(out=sink_t[:, :], in_=sinkr[:, :])

        eps_t = const1.tile([128, 1], F32)
        nc.vector.memset(eps_t[:, :], EPS)
        ones1_t = const1.tile([1, 1], F32)
        nc.vector.memset(ones1_t[:, :], 1.0)
        onescol = const1.tile([S, 1], BF16)
        nc.vector.memset(onescol[:, :], 1.0)
        onesb = const1.tile([C, 1], BF16)
        nc.vector.memset(onesb[:, :], 1.0)

        # ---------------- phase 1: per-patch stats ----------------
        # preload act func set 6 (ln+exp+square+copy) once; the bacc
        # table pass sees it and inserts no further loads
        nc.scalar.add_instruction(mybir.InstLoadActFuncSet(
            name=f"I-{nc.next_id()}", engine=mybir.EngineType.Activation,
            act_func_set_id=6, ins=[], outs=[]))
        # pw broadcast to 128 partitions on the Pool engine
        pwB = const1.tile([128, PATCH], F32)
        nc.gpsimd.partition_broadcast(pwB[:, :], pw_sb[:, :])

        # per-tile pipeline: square+accum (ACT) || dot (DVE) ->
        # rsqrt via ln/exp (ACT) -> logit col (DVE) -> PE transpose ->
        # row chunk copy (ACT); the [1,512] row assembles in SBUF with
        # no SBUF->SBUF DMA.
        ssc = stat.tile([128, 4], F32)
        dotc = stat.tile([128, 4], F32)
        lnc = stat.tile([128, 4], F32)
        rsqc = stat.tile([128, 4], F32)
        lcol = stat.tile([128, 4], F32)
        logits_row = stat.tile([1, NP], F32)
        hstat = stat.tile([128, 4], F32)
        for i in range(4):
            xp = xts[i]
            junk = junkp.tile([128, PATCH], F32, tag="junk")
            junk2 = junkp.tile([128, PATCH], F32, tag="junk2")
            if i < 3:
                nc.scalar.activation(out=junk[:, :], in_=xp[:, :],
                                     func=AF.Square,
                                     accum_out=ssc[:, i:i + 1])
                nc.vector.scalar_tensor_tensor(
                    out=junk2[:, :], in0=xp[:, :], scalar=1.0, in1=pwB[:, :],
                    op0=ALU.mult, op1=ALU.mult, accum_out=dotc[:, i:i + 1])
            else:
                for hf in range(2):
                    sl = slice(1024 * hf, 1024 * (hf + 1))
                    nc.scalar.activation(out=junk[:, sl], in_=xp[:, sl],
                                         func=AF.Square,
                                         accum_out=hstat[:, hf:hf + 1])
                    nc.vector.scalar_tensor_tensor(
                        out=junk2[:, sl], in0=xp[:, sl], scalar=1.0,
                        in1=pwB[:, sl], op0=ALU.mult, op1=ALU.mult,
                        accum_out=hstat[:, 2 + hf:3 + hf])
                nc.vector.tensor_add(out=ssc[:, i:i + 1],
                                       in0=hstat[:, 0:1], in1=hstat[:, 1:2])
                nc.vector.tensor_add(out=dotc[:, i:i + 1],
                                     in0=hstat[:, 2:3], in1=hstat[:, 3:4])
            nc.scalar.activation(out=lnc[:, i:i + 1], in_=ssc[:, i:i + 1],
                                 func=AF.Ln, bias=eps_t[:, 0:1],
                                 scale=1.0 / PATCH)
            nc.scalar.activation(out=rsqc[:, i:i + 1], in_=lnc[:, i:i + 1],
                                 func=AF.Exp, scale=-0.5)
            nc.vector.tensor_mul(lcol[:, i:i + 1], dotc[:, i:i + 1],
                                 rsqc[:, i:i + 1])
            nc.tensor.transpose(out=PS[0:1, 3584 + i * 128:3712 + i * 128],
                                in_=lcol[:, i:i + 1],
                                identity=identf_t[:, :])
        # transpose outputs land contiguous in PSUM; selection reads
        # them there directly (no SBUF row copy)

        if LEVEL == 1:
            nc.sync.dma_start(out=out[:, 0:4], in_=lcol[0:64, :])
            return
        # ---------------- top-4 selection ----------------
        max8 = stat.tile([1, 8], F32)
        nc.vector.max(out=max8[:, :], in_=PS[0:1, 3584:4096])
        idx8 = stat.tile([1, 8], U32)
        nc.vector.max_index(out=idx8[:, :], in_max=max8[:, :],
                            in_values=PS[0:1, 3584:4096])
        # sort the top-4 patch ids ascending: negate, pad low, desc max
        # sort => negated ids ascending; no negative-stride reads
        idxp = stat.tile([1, 8], F32)
        nc.vector.memset(idxp[:, :], -1.0e9)
        nc.vector.tensor_scalar_mul(idxp[:, 0:4], idx8[:, 0:4], -1.0)
        idxs = stat.tile([1, 8], F32)
        nc.vector.max(out=idxs[:, :], in_=idxp[:, :])
        # token order X-grouped: tok[16X+4a+e] = 16*id[a] + 4X + e, so
        # qkvg PSUM rows land grouped by q/k/v/g (per-X flat writes).
        # two 3D STTs (walrus caps STT APs at 3 dims)
        tok0 = stat.tile([1, 16], F32)
        nc.vector.scalar_tensor_tensor(
            out=tok0[:, :],
            in0=rap(idxs, [[8, 1], [1, 4], [0, 4]]),
            scalar=-16.0, in1=off16_t[:, 0:16],
            op0=ALU.mult, op1=ALU.add)
        tok_f = stat.tile([1, NSEL], F32)
        nc.vector.scalar_tensor_tensor(
            out=tok_f[:, :],
            in0=rap(tok0, [[16, 1], [0, 4], [1, 16]]),
            scalar=1.0, in1=off16_t[:, 16:16 + NSEL],
            op0=ALU.mult, op1=ALU.add)
        # gather wants a column of offsets: transpose via K=1 matmul
        nc.tensor.matmul(out=PS[0:NSEL, 3776:3777], lhsT=tok_f[:, :],
                         rhs=ones1_t[:, :], start=True, stop=True)
        tok_i = stat.tile([NSEL, 1], I32)
        nc.vector.tensor_copy(out=tok_i[:, :], in_=PS[0:NSEL, 3776:3777])

        x_sel = sb.tile([NSEL, C], F32, tag="x_sel")
        nc.gpsimd.indirect_dma_start(
            out=x_sel[:, :], out_offset=None, in_=xb[:, :],
            in_offset=bass.IndirectOffsetOnAxis(ap=tok_i[:, 0:1], axis=0))

        if LEVEL == 2:
            nc.sync.dma_start(out=out[:, :], in_=x_sel[:, :])
            return
        # ---------------- qkvg projection (64 tokens, bf16) ----------------
        # warm the PE pstate behind the gather/copy so qkvg runs warm
        nc.tensor.transpose(out=PS[0:128, 3712:3776], in_=x_sel[:, :],
                            identity=identf_t[0:NSEL, 0:NSEL])
        x_selT = sb.tile([128, NSEL], BF16, tag="x_selT")
        nc.scalar.copy(out=x_selT[:, :], in_=PS[0:128, 3712:3776])
        for w in range(8):
            nc.tensor.matmul(out=PS[0:1, 3583:3584], lhsT=x_sel[:, 0:1],
                             rhs=identf_t[0:NSEL, 0:1], start=True, stop=True)

        qkvg_sb = sb.tile([NSEL, FQ], BF16, tag="qkvg")
        for k in range(8):
            nc.tensor.matmul(
                out=PS[0:NSEL, 512 * k:512 * (k + 1)],
                lhsT=x_selT[:, :],
                rhs=wqt_sb[:, 512 * k:512 * (k + 1)],
                start=True, stop=True)
            if k % 2 == 0:
                nc.scalar.copy(out=qkvg_sb[:, 512 * k:512 * (k + 1)],
                               in_=PS[0:NSEL, 512 * k:512 * (k + 1)])
            else:
                nc.vector.tensor_copy(
                    out=qkvg_sb[:, 512 * k:512 * (k + 1)],
                    in_=PS[0:NSEL, 512 * k:512 * (k + 1)])

        # preload -BIG causal bias into the att PSUM region now (banks
        # 4-5 are drained); att matmuls later accumulate on top
        att_v = PS[0:S, 2048:3072].rearrange("t (h c) -> t h c", h=H)[:, :, 0:S]
        nc.vector.tensor_copy(
            out=att_v,
            in_=cm_t[:, :].rearrange("t (a s) -> t a s", a=1)
                .to_broadcast([S, H, S]))

        # permute via DRAM bounce (SBUF DMA APs cross partitions only in
        # dim0): one flat SBUF->DRAM copy, then per-tensor permuted
        # DRAM->SBUF reads. dst partition s = 16a+4e+b reads src row
        # 16a+4X+e at feature block b*1024 -- on the flat DRAM side this
        # collapses to 3 dims [a][e][4096 contiguous].
        qflat = dramp.tile([NSEL, FQ], BF16)
        qkvg_all = sb.tile([S, 4, H, C], BF16, tag="qkvg_all")
        for X, eng in ((0, nc.sync), (1, nc.gpsimd), (3, nc.gpsimd),
                       (2, nc.sync)):
            nc.sync.dma_start(out=qflat[16 * X:16 * (X + 1), :],
                              in_=qkvg_sb[16 * X:16 * (X + 1), :])
            # dst s = 16a+4e+b <- qflat row 16X+4a+e, feature block b*1024
            eng.dma_start(
                out=rap(qkvg_all, [[4 * HC, NSEL], [1, HC]], offset=X * HC),
                in_=rap(qflat[:, :], [[FQ * 4, 4], [FQ, 4], [1, FQ]],
                        offset=X * 16 * FQ))
        # sink row for q/k/v at position partition 64
        nc.scalar.dma_start(out=rap(qkvg_all, [[4 * HC, 1], [1, 3 * HC]],
                                    offset=NSEL * 4 * HC),
                            in_=sink_t[:, :])

        if LEVEL == 3:
            dbg = sb.tile([NSEL, C], F32, tag="dbg3")
            nc.scalar.copy(out=dbg[:, :], in_=qkvg_all[0:NSEL, 0, 0, :])
            nc.sync.dma_start(out=out[:, :], in_=dbg[:, :])
            return
        # ------- transpose q/k first; RoPE + rmsnorm in [c, *] layout ----
        # qkT[c, x, h, s] <- qkvg_all[s, x, h, c]
        PSB = PS[:, :].bitcast(BF16)  # [[8192,128],[1,8192]] bf16 units
        qkT = sb.tile([128, 2, H, S], BF16, tag="qkT")
        for xi in range(2):
            base = 2048 * xi
            for h in range(H):
                nc.tensor.transpose(
                    out=rap(PSB, [[8192, 128], [1, S]],
                            offset=base + h * 256),
                    in_=qkvg_all[:, xi, h, :], identity=identb_t[:, :])
            for hf in range(2):
                src_ap = rap(PSB, [[8192, 128], [256, 4], [1, S]],
                             offset=base + 1024 * hf)
                dst = qkT[:, xi, 4 * hf:4 * hf + 4, :]
                if (xi + hf) % 2 == 0:
                    nc.scalar.copy(out=dst, in_=src_ap)
                else:
                    nc.vector.tensor_copy(out=dst, in_=src_ap)
        # sum of squares per (x, h, s) via PE (rope is a rotation, norms
        # equal): square on ACT, then ones.T @ sq -> [1, 1040] PSUM
        junkT = junkp.tile([128, 2 * H * S], BF16, tag="junkT")
        nc.scalar.activation(out=junkT[:, :],
                             in_=rap(qkT, [[2 * H * S, 128], [1, 2 * H * S]]),
                             func=AF.Square)
        # gating: transpose g on the idle PE, then one sigmoid on the
        # [c, h, s] layout (table swaps hide in ACT's idle window).
        # sink column s=64 is junk and never read (out uses s<64).
        for h in range(H):
            nc.tensor.transpose(
                out=rap(PSB, [[8192, 128], [1, NSEL]],
                        offset=4096 + h * 256),
                in_=qkvg_all[0:NSEL, 3, h, :],
                identity=identb_t[0:NSEL, 0:NSEL])
        gT = sb.tile([128, H, S], BF16, tag="gT")
        nc.scalar.copy(
            out=gT[:, :, 0:NSEL],
            in_=rap(PSB, [[8192, 128], [256, H], [1, NSEL]], offset=4096))
        sigT = sb.tile([128, H, S], BF16, tag="sigT")
        nc.scalar.activation(out=sigT[:, :, :], in_=gT[:, :, :],
                             func=AF.Sigmoid)
        for c0, n in ((0, 512), (512, 512), (1024, 16)):
            nc.tensor.matmul(out=PS[0:1, c0:c0 + n],
                             lhsT=onesb[:, :], rhs=junkT[:, c0:c0 + n],
                             start=True, stop=True)
        lnr = stat.tile([1, 2 * H * S], F32)
        nc.scalar.activation(out=lnr[:, :], in_=PS[0:1, 0:2 * H * S],
                             func=AF.Ln, bias=eps_t[0:1, 0:1],
                             scale=1.0 / C)
        rsq_row = stat.tile([1, 2 * H * S], BF16)
        nc.scalar.activation(out=rsq_row[:, :], in_=lnr[:, :], func=AF.Exp,
                             scale=-0.5)
        # rope in transposed layout: rT = qkT*cosT + shift128(qkT)*sinT
        rT = sb.tile([128, 2, H, S], BF16, tag="rT")
        rTf = rap(rT, [[2 * H * S, 128], [1, 2 * H * S]])
        cosT_b = rap(cosT_t, [[S, 128], [0, 2 * H], [1, S]])
        nc.vector.tensor_tensor(
            out=rap(rT, [[2 * H * S, 128], [S, 2 * H], [1, S]]),
            in0=rap(qkT, [[2 * H * S, 128], [S, 2 * H], [1, S]]),
            in1=cosT_b, op=ALU.mult)
        # DVE lanes cannot read shifted partitions: build the half-rotated
        # copy with two SBUF->SBUF DMAs (hidden under the norm branch)
        qkT_sw = sb.tile([128, 2, H, S], BF16, tag="qkT_sw")
        F1 = 2 * H * S
        nc.sync.dma_start(
            out=rap(qkT_sw, [[F1, 64], [1, F1]]),
            in_=rap(qkT, [[F1, 64], [1, F1]], offset=64 * F1))
        nc.sync.dma_start(
            out=rap(qkT_sw, [[F1, 64], [1, F1]], offset=64 * F1),
            in_=rap(qkT, [[F1, 64], [1, F1]]))
        r2T = sb.tile([128, 2, H, S], BF16, tag="r2T")
        nc.vector.tensor_tensor(
            out=rap(r2T, [[F1, 128], [1, F1]]),
            in0=rap(qkT_sw, [[F1, 128], [1, F1]]),
            in1=rap(sinT_t, [[S, 128], [0, 2 * H], [1, S]]),
            op=ALU.mult)
        nc.vector.tensor_add(out=rTf, in0=rTf,
                             in1=rap(r2T, [[2 * H * S, 128], [1, 2 * H * S]]))
        # normalize: broadcast rsq across partitions on Pool, one DVE mult
        rsqB = sb.tile([128, 2 * H * S], BF16, tag="rsqB")
        nc.gpsimd.partition_broadcast(rsqB[:, :], rsq_row[:, :])
        qknT = sb.tile([128, 2, H, S], BF16, tag="qknT")
        nc.vector.tensor_tensor(
            out=rap(qknT, [[2 * H * S, 128], [1, 2 * H * S]]),
            in0=rTf, in1=rsqB[:, :], op=ALU.mult)
        qnT = qknT[:, 0, :, :]
        knT = qknT[:, 1, :, :]

        # attention computed TRANSPOSED: attT[t, h, s] via swapped
        # operands, so exp output pmT feeds the y matmul as lhsT directly
        # (no softmax transpose pass); bias was preloaded above.
        for h in range(H):
            nc.tensor.matmul(out=PS[0:S, 2048 + h * 128:2048 + h * 128 + S],
                             lhsT=knT[:, h, :], rhs=qnT[:, h, :],
                             start=False, stop=True, skip_group_check=True)
        # pT = exp((attT + bias) * tao0*tao1*SCALE)   (no max-sub needed)
        pT = sb.tile([S, H, S], BF16, tag="pT")
        nc.scalar.activation(out=pT[:, :, :], in_=att_v, func=AF.Exp,
                             scale=tsc_t[:, 0:1])
        # den rows [1, h, s] via ones.T @ pT; y TRANSPOSED via v.T @ pT
        # so the gate/out-projection never transpose y
        for h in range(H):
            nc.tensor.matmul(out=PS[0:1, 3072 + h * 128:3072 + h * 128 + S],
                             lhsT=onescol[:, :], rhs=pT[:, h, :],
                             start=True, stop=True)
        rden_row = stat.tile([1, H, S], F32)
        nc.vector.reciprocal(
            out=rden_row[:, :, :],
            in_=PS[0:1, 3072:4096].rearrange("p (h c) -> p h c", h=H)[:, :, 0:S])
        rdenB = sb.tile([128, H, S], BF16, tag="rdenB")
        nc.gpsimd.partition_broadcast(
            rdenB[:, :, :], rden_row[:, :, :])
        for h in range(H):
            nc.tensor.matmul(out=PS[0:128, h * 128:h * 128 + S],
                             lhsT=qkvg_all[:, 2, h, :], rhs=pT[:, h, :],
                             start=True, stop=True)
        # yT [c, h, s] in PSUM; gate with sigT and 1/den
        ygT = sb.tile([128, H, S], BF16, tag="ygT")
        nc.vector.tensor_tensor(
            out=ygT[:, :, :],
            in0=PS[:, 0:1024].rearrange("c (h s) -> c h s", h=H)[:, :, 0:S],
            in1=sigT[:, :, :], op=ALU.mult)
        nc.vector.tensor_tensor(out=ygT[:, :, :], in0=ygT[:, :, :],
                                in1=rdenB[:, :, :], op=ALU.mult)
        # ---------------- output projection ----------------
        out_sb = sb.tile([NSEL, C], F32, tag="out_sb")
        for cf in range(2):
            for h in range(H):
                nc.tensor.matmul(
                    out=PS[0:NSEL, 2048 + 64 * cf:2112 + 64 * cf],
                    lhsT=ygT[:, h, 0:NSEL],
                    rhs=wot_sb[:, h, 64 * cf:64 * (cf + 1)],
                    start=(h == 0), stop=(h == H - 1))
            if cf == 0:
                nc.scalar.copy(out=out_sb[:, 0:64],
                               in_=PS[0:NSEL, 2048:2112])
            else:
                nc.vector.tensor_copy(out=out_sb[:, 64:128],
                                      in_=PS[0:NSEL, 2112:2176])
            nc.sync.dma_start(out=out[:, 64 * cf:64 * (cf + 1)],
                              in_=out_sb[:, 64 * cf:64 * (cf + 1)])


def make_host_constants():
    identf = np.eye(C, dtype=np.float32)
    identb = np.eye(S, dtype=BF)
    m = np.arange(NSEL)
    off16r = np.concatenate([np.arange(16) % 4,
                             (m // 16) * 4]).astype(np.float32).reshape(1, 80)
    negio = (float(NP) - np.arange(NP, dtype=np.float32)).reshape(1, NP)
    # partition p < 64 holds sequence position p+1; partition 64 is the
    # sink (position 0); multiplicative causal mask
    pos = np.where(np.arange(S) < NSEL, np.arange(S) + 1, 0)
    cm01 = np.ascontiguousarray(
        np.where(pos[None, :] <= pos[:, None], 0.0,
                 -1e30).astype(np.float32).T)
    return identf, identb, off16r, negio, cm01


_CACHE = {}


def get_nc():
    if "nc" not in _CACHE:
        nc = bacc.Bacc("TRN2", target_bir_lowering=False, debug=False,
                       num_devices=B)
        build_kernel(nc)
        nc.compile()
        _CACHE["nc"] = nc
    return _CACHE["nc"]


def make_in_maps(inputs):
    x = np.ascontiguousarray(inputs["x"], dtype=np.float32)
    cos = np.asarray(inputs["cos"]).reshape(S, 64).astype(np.float32)
    sin = np.asarray(inputs["sin"]).reshape(S, 64).astype(np.float32)
    # permute rows: partition p = seq position p+1 (p<64), partition 64 = pos 0
    perm = np.concatenate([np.arange(1, S), [0]])
    cp, sp = cos[perm], sin[perm]                     # [65, 64]
    cosb = np.ascontiguousarray(
        np.concatenate([cp.T, cp.T], axis=0)).astype(BF)       # [128, 65]
    sindb = np.ascontiguousarray(
        np.concatenate([sp.T, -sp.T], axis=0)).astype(BF)      # [128, 65]
    sinkv = np.asarray(inputs["sink"]).reshape(1, HC).astype(np.float32)
    sinkr = np.ascontiguousarray(
        np.broadcast_to(sinkv, (3, HC)).reshape(1, 3 * HC)).astype(BF)
    wqkvg = np.asarray(inputs["W_qkvg"], dtype=np.float32)
    wqt = np.ascontiguousarray(wqkvg.T).astype(BF)                # [128, 4096]
    wout = np.asarray(inputs["W_out"], dtype=np.float32)          # [128, 1024]
    # wot[c, h, co] = W_out[co, h*128+c]
    wot = np.ascontiguousarray(
        wout.T.reshape(H, C, C).transpose(1, 0, 2).reshape(C, HC)).astype(BF)
    pwv = np.asarray(inputs["patch_w"], dtype=np.float32).reshape(1, PATCH)
    tao = np.asarray(inputs["tao"], dtype=np.float32)
    tscv = np.full((S, 1), float(tao[0]) * float(tao[1]) * SCALE,
                   dtype=np.float32)
    identf, identb, off16r, negio, cm01 = make_host_constants()
    in_maps = []
    for b in range(B):
        in_maps.append({
            "xb": np.ascontiguousarray(x[b]),
            "pw": pwv, "wqt": wqt, "wot": wot, "sinkr": sinkr,
            "cosb": cosb, "sindb": sindb, "cm01": cm01, "negio": negio,
            "off16r": off16r, "tsc": tscv, "identf": identf,
            "identb": identb,
        })
    return in_maps


def kernel(**inputs):
    nc = get_nc()
    in_maps = make_in_maps(inputs)
    res = run_bass_kernel_spmd(nc, in_maps, core_ids=list(range(B)))
    return np.stack([r["out"] for r in res.results], axis=0)


if __name__ == "__main__":
    nc = get_nc()
    print("build ok:", len(nc.m.functions[0].allocations), "allocations")
